# revision 25
# baseline (speedup 1.0000x reference)
"""Trainium2 Bass kernel for LocalWindowAttention — v4 engine-balanced.

Per batch b (one NeuronCore):
    tokens = xb[b].mean(-1)                    # [NB, C]
    Q/K/V  = tokens @ W{q,k,v} + pos           # [NB, D]
    scores = window-attn over NB (win=9, clamped) with scale 1/sqrt(D)
    ctx    = softmax(scores) @ V_window        # [NB, D]
    out    = xb[b] + (ctx @ Wo)[..., None]     # broadcast over T, residual

Key ideas vs v3 (533 us measured same-day, DMA idle ~127 us in
per-slot gaps; v4 measures 487-493 us, rel err 1.0e-3):
1. Single xb read, t-major [NB, T, C] fp16 device layout, fp8 weights
   with DoubleRow matmuls, fused (Wv/T)@Wo — all kept from v3.
2. The v3 trace showed DVE at 47.9 us/block vs the 46.8 us/block DMA
   budget, and the attention chain threaded through the (in-order,
   backlogged) DVE queue.  v4 rebalances:
   - mask add and pos@Wo add ride the PE as identity matmuls
     accumulated straight into the scores/U psum (stream the mask/pos
     tile against an identity stationary) — the DVE TT is gone and
     exp reads psum directly.
   - softmax normalization is deferred: attn_out evicts the
     UNNORMALIZED den*out_tok; the next slot's phase_b does
     rden = 1/den (DVE, front-of-queue) + one ACT per-partition-scale
     copy.  The attention chain never waits on the DVE queue.
   - NPE=3 of 8 residual chunks per block run on PE+ACT (two identity
     matmuls into psum: xb-slice + otok-slice, ACT evicts back over
     the stream tile); the other 5 stay on the DVE.  NPE=4 measured
     WORSE (506 us): the ACT queue becomes the slot-boundary gate.
   Engine busy/block: DVE 31.6, ACT 31.9, PE 29.6 vs DMA 46.8.
3. Emission reorders: block-0 chunk loads issue before the weight
   loads (sync-ring FIFO); the epilogue runs attention(7) before the
   trailing phase_b(6)/phase_b(7), and phase_b(7) borrows the free
   otok buffer for its normalized-otok so it never serializes behind
   phase_b(6)'s DVE TTs (tmp pool has 1 buffer).
Rejected experiments (measured): bf16 streams — no speedup (the DVE
TT rate ~1.6-1.7 elem/cyc is dtype-independent), 2.5x worse error.
Remaining known bottleneck: a per-slot latency cascade
(last-load -> reduce-tail -> K-proj -> attention -> otok -> residual
TTs -> stores -> buffer frees -> loads) spills ~8-13 us of DMA idle
per 47 us slot; splitting attention into a "left" part (computable
one slot early) plus a 4x4 window corner would shave ~5 us/slot more.
Quantization: out_tok carries ~5-7% rms error but is only ~7% of
|out|; end-to-end rel err ~1.0e-3 << the 2e-2 gate.
"""

import numpy as np
import ml_dtypes

import concourse.bass as bass
import concourse.mybir as mybir
import concourse.tile as tile
import concourse.bacc as bacc
from concourse import masks as cmasks
from concourse.bass_utils import run_bass_kernel_spmd

# Problem shapes (hardcoded per contest rules)
B, NB, C, T = 8, 1024, 1024, 32
D = 1024
WIN, HALF = 9, 4
P = 128                       # partitions
NBLK = NB // P                # 8 row blocks
CCH = C // P                  # 8 c-chunks
DCH = D // P                  # 8 d-chunks
WWIN = 192                    # window columns per block (32-aligned segs)
SCALE = 1.0 / np.sqrt(D)      # 1/32
TC = 4                        # t-rows per stream chunk
NCH = T // TC                 # chunks per block (8)
NSTREAM = 19                  # stream pool buffers (2 blocks + 3 spares)
NPE = 3                       # residual chunks offloaded to PE+ACT

F32 = mybir.dt.float32
BF16 = mybir.dt.bfloat16
F16 = mybir.dt.float16
FP8 = mybir.dt.float8e4
NPF8 = ml_dtypes.float8_e4m3
DR = mybir.MatmulPerfMode.DoubleRow

# fp8 scale plumbing (see module docstring)
S_TOK = 1.0 / 16.0            # tokens (T-sums, std ~5.7) -> fp8
S_W = 128.0                   # projection weights W/T -> fp8
S_QK = S_TOK * S_W            # = 8: Q/K/V psum pre-scale
S_WVO = 128.0                 # fused (Wv/T)@Wo -> fp8

N_CORES = 8


def _w0(i):
    """Window start for block i; all V-block segments 32-aligned."""
    return min(max(i * P - 32, 0), NB - WWIN)


def _build_masks():
    """Per-block additive masks [NBLK, P, WWIN], pre-scaled for the
    exp (which uses scale=SCALE/S_QK^2): log(multiplicity) *
    S_QK^2/SCALE on in-band columns, -1e30 elsewhere."""
    m = np.full((NBLK, P, WWIN), -1e30, np.float32)
    for i in range(NBLK):
        w0 = _w0(i)
        for r in range(P):
            n = i * P + r
            idx = np.clip(n - HALF + np.arange(WIN), 0, NB - 1)
            u, cnt = np.unique(idx, return_counts=True)
            m[i, r, u - w0] = np.log(cnt.astype(np.float64)) * (
                S_QK * S_QK / SCALE)
    return m


_MASKS64 = _build_masks()


def _segments(i):
    """V-block segments covering window [w0, w0+WWIN) for block i as
    (blk, p0, ln, cofs): rows [p0, p0+ln) of V block `blk` correspond
    to window columns [cofs, cofs+ln).  All splits 32-aligned."""
    w0 = _w0(i)
    segs = []
    lo, hi = w0, w0 + WWIN
    for blk in range(NBLK):
        b0, b1 = blk * P, (blk + 1) * P
        s0, s1 = max(lo, b0), min(hi, b1)
        if s0 < s1:
            segs.append((blk, s0 - b0, s1 - s0, s0 - w0))
    return segs


def build_nc():
    nc = bacc.Bacc("TRN2", target_bir_lowering=False, debug=False,
                   num_devices=N_CORES)

    xb_h = nc.declare_dram_parameter("xbh", [NB, T, C], F16, isOutput=False)
    wq_d = nc.declare_dram_parameter("wq", [C, D], FP8, isOutput=False)
    wk_d = nc.declare_dram_parameter("wk", [C, D], FP8, isOutput=False)
    wvo_d = nc.declare_dram_parameter("wvo", [C, C], FP8, isOutput=False)
    posu_d = nc.declare_dram_parameter("posu", [NB, C], FP8, isOutput=False)
    mask_d = nc.declare_dram_parameter("mask", [NBLK, P, WWIN], BF16,
                                       isOutput=False)
    out_d = nc.declare_dram_parameter("out", [NB, T, C], F16, isOutput=True)

    with tile.TileContext(nc) as tc:
        _emit(nc, tc, xb_h, wq_d, wk_d, wvo_d, posu_d,
              mask_d, out_d)

    nc.compile()
    return nc


def _emit(nc, tc, xb_h, wq_d, wk_d, wvo_d, posu_d,
          mask_d, out_d):
    from contextlib import ExitStack

    with ExitStack() as ctx:
        const_pool = ctx.enter_context(tc.tile_pool(name="const", bufs=1))
        ident = const_pool.tile([P, P], BF16)
        cmasks.make_identity(nc, ident[:])
        identF = const_pool.tile([P, P], F16, tag="idF")
        cmasks.make_identity(nc, identF[:])
        ident8 = const_pool.tile([P, P], FP8, tag="id8")
        cmasks.make_identity(nc, ident8[:])

        persist = ctx.enter_context(tc.tile_pool(name="persist", bufs=1))
        kT = persist.tile([P, DCH, NB], FP8)         # K^T (d-major), *8
        # U = (V@Wo) (n-major, *8), rolling window of 4 blocks: lets
        # attention finish with a single attn^T @ U matmul instead of
        # the two-stage (attn@V)@Wo, shortening the per-slot critical
        # chain by ~6us
        uA = persist.tile([P, 4, C], FP8)

        wpool = ctx.enter_context(tc.tile_pool(name="weights", bufs=1))
        wq = wpool.tile([P, CCH, D], FP8)
        wk = wpool.tile([P, CCH, D], FP8)
        wvo = wpool.tile([P, CCH, C], FP8)

        stream = ctx.enter_context(tc.tile_pool(name="stream", bufs=NSTREAM))
        scr_pool = ctx.enter_context(tc.tile_pool(name="scratch", bufs=1))
        tok_pool = ctx.enter_context(tc.tile_pool(name="tokens", bufs=1))
        tokT_pool = ctx.enter_context(tc.tile_pool(name="tokT", bufs=2))
        mask_pool = ctx.enter_context(tc.tile_pool(name="mask", bufs=2))
        qT_pool = ctx.enter_context(tc.tile_pool(name="qT", bufs=2))
        pos_pool = ctx.enter_context(tc.tile_pool(name="pos", bufs=1))
        otok_pool = ctx.enter_context(tc.tile_pool(name="otok", bufs=2))
        tmp_pool = ctx.enter_context(tc.tile_pool(name="otokn", bufs=1))
        att_pool = ctx.enter_context(tc.tile_pool(name="attn", bufs=1))

        # PSUM: 8 banks -> psTR 1, ps512 2, ps128 2, psS 1, psR 2
        psTR = ctx.enter_context(
            tc.tile_pool(name="psTR", bufs=1, space="PSUM"))  # transposes
        ps512 = ctx.enter_context(
            tc.tile_pool(name="ps512", bufs=2, space="PSUM"))  # V/out_tok
        ps128 = ctx.enter_context(
            tc.tile_pool(name="ps128", bufs=2, space="PSUM"))  # QK/ctx
        psS = ctx.enter_context(
            tc.tile_pool(name="psS", bufs=1, space="PSUM"))   # scores
        psR = ctx.enter_context(
            tc.tile_pool(name="psR", bufs=2, space="PSUM"))   # PE residual

        def issue_loads(i):
            """Queue block i's 8 chunk loads on the SP HWDGE ring."""
            tiles = []
            for j in range(NCH):
                t = stream.tile([P, TC, C], F16, tag="xb")
                nc.sync.dma_start(
                    t[:], xb_h[i * P:(i + 1) * P, j * TC:(j + 1) * TC, :])
                tiles.append(t)
            return tiles

        def reduce_project(i, tiles):
            """T-sum block i (kept resident), transpose, project K
            (which gates attention(i-1))."""
            tok = tok_pool.tile([P, C], BF16)
            for j, t in enumerate(tiles):
                # all-2x reduction tree: out-of-place first level keeps
                # the raw xb intact for the phase-B residual
                s = scr_pool.tile([P, 2, C], F16, tag="s")
                nc.vector.tensor_tensor(
                    s[:], t[:, 0:2, :], t[:, 2:4, :],
                    op=mybir.AluOpType.add)
                with nc.allow_low_precision("tokens feed fp8 matmuls"):
                    if j == 0:
                        nc.vector.tensor_tensor(
                            tok[:], s[:, 0, :], s[:, 1, :],
                            op=mybir.AluOpType.add)
                    else:
                        nc.vector.tensor_tensor(
                            s[:, 0, :], s[:, 0, :], s[:, 1, :],
                            op=mybir.AluOpType.add)
                        nc.vector.tensor_tensor(
                            tok[:], tok[:], s[:, 0, :],
                            op=mybir.AluOpType.add)
            tokT = tokT_pool.tile([P, CCH, P], FP8)
            for cc in range(CCH):
                pt = psTR.tile([P, P], BF16, tag="tr")
                nc.tensor.transpose(pt[:], tok[:, cc * P:(cc + 1) * P],
                                    ident[:])
                nc.scalar.activation(tokT[:, cc, :], pt[:],
                                     mybir.ActivationFunctionType.Copy,
                                     scale=float(S_TOK))

            ic = slice(i * P, (i + 1) * P)
            posu8 = pos_pool.tile([P, C], FP8, tag="posu")
            nc.sync.dma_start(
                posu8[:], posu_d.rearrange("(a p) c -> p a c", p=P)[:, i, :])
            mrow = mask_pool.tile([P, WWIN], BF16, tag="m")
            nc.sync.dma_start(
                mrow[:], mask_d.rearrange("a p w -> p a w")[:, i, :])

            # K first: attention(i-1) waits on it, so this sits on the
            # per-slot critical cascade.  Plain fp8 (no DoubleRow):
            # FWL loads the 128-col stationary in ~40ns vs DR's ~218ns
            # reload per 256-row pass — at FD=128 (DR's break-even)
            # the LDW saving wins and the cascade's K segment shrinks.
            # pos is dropped from Q/K: its score contribution (~0.0004
            # in exp units) is far below the fp8 noise floor.
            for dd in range(DCH):
                ps = ps128.tile([P, P], F32, tag="qk")
                for cc in range(CCH):
                    nc.tensor.matmul(
                        ps[:], wk[:, cc, dd * P:(dd + 1) * P],
                        tokT[:, cc, :],
                        start=(cc == 0), stop=(cc == CCH - 1))
                nc.scalar.copy(kT[:, dd, ic], ps[:])
            return tokT, posu8, mrow

        def proj_u(i, tokT, posu8):
            """U(i) = tokens@((Wv/T)@Wo) + pos@Wo, fused on the host
            into one [C, C] weight — psum arrives at *8 like Q/K; the
            pos@Wo term is accumulated into the psum by an fp8
            identity matmul streaming posu, and ACT evicts to fp8.
            Emitted between attention's softmax and its out_tok matmul
            so the PE fills the softmax latency."""
            for ch in range(C // 512):
                cs = slice(ch * 512, (ch + 1) * 512)
                ups = ps512.tile([P, 512], F32, tag="v")
                for cc in range(0, CCH, 2):
                    nc.tensor.matmul(ups[:], tokT[:, cc:cc + 2, :],
                                     wvo[:, cc:cc + 2, cs],
                                     start=(cc == 0), stop=False,
                                     perf_mode=DR)
                nc.tensor.matmul(ups[:], ident8[:], posu8[:, cs],
                                 start=False, stop=True)
                with nc.allow_low_precision("uA feeds fp8 matmuls"):
                    nc.scalar.copy(uA[:, i % 4, cs], ups[:])

        def proj_q(i, tokT):
            """Q projection for block i (needed only by attention(i),
            so emitted after attention(i-1) to keep PE unblocked)."""
            qTb = qT_pool.tile([P, DCH, P], FP8)
            for dd in range(DCH):
                ps = ps128.tile([P, P], F32, tag="qk")
                for cc in range(0, CCH, 2):
                    nc.tensor.matmul(
                        ps[:], wq[:, cc:cc + 2, dd * P:(dd + 1) * P],
                        tokT[:, cc:cc + 2, :],
                        start=(cc == 0), stop=(cc == CCH - 2),
                        perf_mode=DR)
                nc.scalar.copy(qTb[:, dd, :], ps[:])
            return qTb

        def attn_scores(x, qTb, mrow):
            """Banded scores + softmax + transposed attn for block x.
            The additive mask rides the PE: an identity-stationary
            matmul streams the mask row into the scores psum, so exp
            reads psum directly and the DVE is never involved."""
            w0 = _w0(x)
            segs = _segments(x)

            sc = psS.tile([P, WWIN], F32)
            for dd in range(0, DCH, 2):
                nc.tensor.matmul(sc[:], qTb[:, dd:dd + 2, :],
                                 kT[:, dd:dd + 2, w0:w0 + WWIN],
                                 start=(dd == 0), stop=False,
                                 perf_mode=DR)
            nc.tensor.matmul(sc[:], ident[:], mrow[:],
                             start=False, stop=True)
            # unnormalized exp straight to bf16; the softmax
            # denominator is applied NEXT slot in phase_b (rden recip
            # + ACT scale-copy), so this chain never queues on DVE
            attb = att_pool.tile([P, WWIN], BF16, tag="attb")
            den = att_pool.tile([P, 1], F32, tag="den", bufs=3)
            with nc.allow_low_precision("unnormalized attn weights"):
                nc.scalar.activation(attb[:], sc[:],
                                     mybir.ActivationFunctionType.Exp,
                                     scale=float(SCALE / (S_QK * S_QK)),
                                     accum_out=den[:])

            # transpose attn segments (32-aligned partition placement)
            attT = []
            for (blk, p0, ln, cofs) in segs:
                pt = psTR.tile([P, P], BF16, tag="tr")
                nc.tensor.transpose(pt[p0:p0 + ln, :],
                                    attb[:, cofs:cofs + ln], ident[:],
                                    tile_position=(0, p0))
                st = att_pool.tile([P, P], FP8, tag="attT_sb", bufs=3)
                nc.scalar.copy(st[p0:p0 + ln, :], pt[p0:p0 + ln, :])
                attT.append(st)
            return attT, den

        def attn_out(x, attT):
            """otok_un = den*out_tok: attn^T-weighted sum of U rows;
            the psum is 8*den*out_tok, evicted at 1/8 — normalization
            by 1/den happens in phase_b."""
            segs = _segments(x)
            otok = otok_pool.tile([P, C], F16, tag="otok")
            for ch in range(C // 512):
                cs = slice(ch * 512, (ch + 1) * 512)
                ops = ps512.tile([P, 512], F32, tag="v")
                for k, (blk, p0, ln, cofs) in enumerate(segs):
                    nc.tensor.matmul(
                        ops[:], attT[k][p0:p0 + ln, :],
                        uA[p0:p0 + ln, blk % 4, cs],
                        start=(k == 0), stop=(k == len(segs) - 1),
                        tile_position=(p0, 0))
                with nc.allow_low_precision("den*out_tok fits fp16"):
                    nc.scalar.activation(otok[:, cs], ops[:],
                                         mybir.ActivationFunctionType.Copy,
                                         scale=float(1.0 / S_QK))
            return otok

        def phase_b(x, tiles, otok, den, final=False):
            """Residual broadcast-add on the resident xb chunks of
            block x, then fp16 store (ACT HWDGE ring so stores never
            head-of-line-block the SP-ring loads).  First normalize:
            rden=1/den on DVE (front of the slot's DVE queue), one ACT
            per-partition-scale copy -> tmp.  Chunks 0..NPE-1 run on
            PE+ACT (identity matmuls into psum, ACT evicts in place);
            the rest are DVE broadcast-adds on the 2x path."""
            rden = att_pool.tile([P, 1], F32, tag="rden", bufs=2)
            nc.vector.reciprocal(rden[:], den[:])
            # the single tmp buffer's next grab waits on the PREVIOUS
            # phase_b's slow readers (DVE TTs); in the epilogue, borrow
            # the otok pool's free buffer instead so phase_b(7) doesn't
            # serialize behind phase_b(6)
            if final:
                tmp = otok_pool.tile([P, C], F16, tag="otok")
            else:
                tmp = tmp_pool.tile([P, C], F16)
            with nc.allow_low_precision("normalized out_tok"):
                nc.scalar.activation(tmp[:], otok[:],
                                     mybir.ActivationFunctionType.Copy,
                                     scale=rden[:])
            for j in range(NCH):
                t = tiles[j]
                if j < NPE:
                    for s8 in range(8):
                        tt, chh = divmod(s8, 2)
                        sl = slice(chh * 512, (chh + 1) * 512)
                        ps = psR.tile([P, 512], F32)
                        nc.tensor.matmul(ps[:], identF[:], t[:, tt, sl],
                                         start=True, stop=False)
                        nc.tensor.matmul(ps[:], identF[:], tmp[:, sl],
                                         start=False, stop=True)
                        nc.scalar.copy(t[:, tt, sl], ps[:])
                else:
                    nc.vector.tensor_tensor(
                        t[:], t[:],
                        tmp.unsqueeze(1).broadcast_to((P, TC, C)),
                        op=mybir.AluOpType.add)
                nc.scalar.dma_start(
                    out_d[x * P:(x + 1) * P, j * TC:(j + 1) * TC, :], t[:])

        # Software pipeline, phase B lagged TWO slots behind the loads:
        # at slot i the stores of block i-2 (whose out_tok was computed
        # during slot i-1) begin immediately, so the DMA engines never
        # wait on the current slot's attention chain.
        tiles0 = issue_loads(0)
        # weights ride the same SP ring AFTER block 0's data: xb bytes
        # start flowing at t=0; the weights land ~30us in, well before
        # their first use
        nc.sync.dma_start(wq[:], wq_d.rearrange("(a p) d -> p a d", p=P))
        nc.sync.dma_start(wk[:], wk_d.rearrange("(a p) d -> p a d", p=P))
        nc.sync.dma_start(wvo[:], wvo_d.rearrange("(a p) c -> p a c", p=P))

        prev = None        # (x, tiles, qTb, mrow): awaiting attention
        pend = None        # (x, tiles, otok, den): awaiting phase B
        for i in range(NBLK):
            if pend is not None:
                phase_b(*pend)
                pend = None
            tiles_i = tiles0 if i == 0 else issue_loads(i)
            tokT_i, posu8_i, mrow_i = reduce_project(i, tiles_i)
            if prev is not None:
                x, tiles_x, qTb_x, mrow_x = prev
                attT_x, den_x = attn_scores(x, qTb_x, mrow_x)
                proj_u(i, tokT_i, posu8_i)   # PE fills softmax latency
                otok_x = attn_out(x, attT_x)
                pend = (x, tiles_x, otok_x, den_x)
            else:
                proj_u(i, tokT_i, posu8_i)
            qTb_i = proj_q(i, tokT_i)
            prev = (i, tiles_i, qTb_i, mrow_i)
        # epilogue: kick off attention(7) BEFORE draining the pending
        # stores — block 7's window is clamped inside blocks 6..7, so
        # nothing new is needed and the tail overlaps
        x, tiles_x, qTb_x, mrow_x = prev
        attT_x, den_x = attn_scores(x, qTb_x, mrow_x)
        otok_x = attn_out(x, attT_x)
        if pend is not None:
            phase_b(*pend)
        phase_b(x, tiles_x, otok_x, den_x, final=True)


_NC = None


def _get_nc():
    global _NC
    if _NC is None:
        _NC = build_nc()
    return _NC


def _prep_in_maps(xb, Wq, Wk, Wv, Wo, pos):
    # device works t-major: [NB, T, C]
    xb_h = np.asarray(xb, np.float32).transpose(0, 1, 3, 2).astype(
        np.float16)
    wq8 = (np.asarray(Wq, np.float32) * (S_W / T)).astype(NPF8)
    wk8 = (np.asarray(Wk, np.float32) * (S_W / T)).astype(NPF8)
    wvo = (np.asarray(Wv, np.float32) / T) @ np.asarray(Wo, np.float32)
    wvo8 = (wvo * S_WVO).astype(NPF8)
    posu8 = ((np.asarray(pos, np.float32) @ np.asarray(Wo, np.float32))
             * S_QK).astype(NPF8)
    mask_h = _MASKS64.astype(ml_dtypes.bfloat16)
    in_maps = []
    for b in range(B):
        in_maps.append({
            "xbh": np.ascontiguousarray(xb_h[b]),
            "wq": wq8, "wk": wk8, "wvo": wvo8,
            "posu": posu8, "mask": mask_h,
        })
    return in_maps


def _post(res):
    out = np.stack([res.results[b]["out"] for b in range(B)], axis=0)
    # [B, NB, T, C] fp16 -> [B, NB, C, T] fp32
    return np.ascontiguousarray(out.transpose(0, 1, 3, 2)).astype(
        np.float32)


def kernel(xb, Wq, Wk, Wv, Wo, pos):
    nc = _get_nc()
    in_maps = _prep_in_maps(xb, Wq, Wk, Wv, Wo, pos)
    res = run_bass_kernel_spmd(nc, in_maps, core_ids=list(range(N_CORES)))
    return _post(res)


def run_profiled(xb, Wq, Wk, Wv, Wo, pos, **kw):
    """Like kernel(), but NTFF-profiled; returns (out, BassKernelResults)."""
    import sys, types
    if "antenv.axon_hooks" not in sys.modules:
        try:
            from trn_agent_boot.trn_boot import _ntff_profile_via_ctypes
            hook = _ntff_profile_via_ctypes('/opt/axon/libaxon_pjrt.so')
            mod = types.ModuleType("antenv.axon_hooks")
            mod.get_axon_ntff_profile_hook = lambda: hook
            mod.set_axon_ntff_profile_hook = lambda h: None
            sys.modules["antenv.axon_hooks"] = mod
            import concourse.bass_utils as bu
            bu.upload_artifacts = lambda tmpdir: f"local:{tmpdir}"
        except Exception as e:
            print(f"profiling shim unavailable: {e}")
    nc = _get_nc()
    in_maps = _prep_in_maps(xb, Wq, Wk, Wv, Wo, pos)
    res = run_bass_kernel_spmd(nc, in_maps, core_ids=list(range(N_CORES)),
                               trace=True, **kw)
    return _post(res), res


# revision 26
# speedup vs baseline: 1.1098x; 1.1098x over previous
"""Trainium2 Bass kernel for LocalWindowAttention — v4 engine-balanced.

Per batch b (one NeuronCore):
    tokens = xb[b].mean(-1)                    # [NB, C]
    Q/K/V  = tokens @ W{q,k,v} + pos           # [NB, D]
    scores = window-attn over NB (win=9, clamped) with scale 1/sqrt(D)
    ctx    = softmax(scores) @ V_window        # [NB, D]
    out    = xb[b] + (ctx @ Wo)[..., None]     # broadcast over T, residual

Key ideas vs v3 (533 us measured same-day, DMA idle ~127 us in
per-slot gaps; v4 measures 487-493 us, rel err 1.0e-3):
1. Single xb read, t-major [NB, T, C] fp16 device layout, fp8 weights
   with DoubleRow matmuls, fused (Wv/T)@Wo — all kept from v3.
2. The v3 trace showed DVE at 47.9 us/block vs the 46.8 us/block DMA
   budget, and the attention chain threaded through the (in-order,
   backlogged) DVE queue.  v4 rebalances:
   - mask add and pos@Wo add ride the PE as identity matmuls
     accumulated straight into the scores/U psum (stream the mask/pos
     tile against an identity stationary) — the DVE TT is gone and
     exp reads psum directly.
   - softmax normalization is deferred: attn_out evicts the
     UNNORMALIZED den*out_tok; the next slot's phase_b does
     rden = 1/den (DVE, front-of-queue) + one ACT per-partition-scale
     copy.  The attention chain never waits on the DVE queue.
   - NPE=3 of 8 residual chunks per block run on PE+ACT (two identity
     matmuls into psum: xb-slice + otok-slice, ACT evicts back over
     the stream tile); the other 5 stay on the DVE.  NPE=4 measured
     WORSE (506 us): the ACT queue becomes the slot-boundary gate.
   Engine busy/block: DVE 31.6, ACT 31.9, PE 29.6 vs DMA 46.8.
3. Emission reorders: block-0 chunk loads issue before the weight
   loads (sync-ring FIFO); the epilogue runs attention(7) before the
   trailing phase_b(6)/phase_b(7), and phase_b(7) borrows the free
   otok buffer for its normalized-otok so it never serializes behind
   phase_b(6)'s DVE TTs (tmp pool has 1 buffer).
Rejected experiments (measured): bf16 streams — no speedup (the DVE
TT rate ~1.6-1.7 elem/cyc is dtype-independent), 2.5x worse error.
Remaining known bottleneck: a per-slot latency cascade
(last-load -> reduce-tail -> K-proj -> attention -> otok -> residual
TTs -> stores -> buffer frees -> loads) spills ~8-13 us of DMA idle
per 47 us slot; splitting attention into a "left" part (computable
one slot early) plus a 4x4 window corner would shave ~5 us/slot more.
Quantization: out_tok carries ~5-7% rms error but is only ~7% of
|out|; end-to-end rel err ~1.0e-3 << the 2e-2 gate.
"""

import numpy as np
import ml_dtypes

import concourse.bass as bass
import concourse.mybir as mybir
import concourse.tile as tile
import concourse.bacc as bacc
from concourse import masks as cmasks
from concourse.bass_utils import run_bass_kernel_spmd

# Problem shapes (hardcoded per contest rules)
B, NB, C, T = 8, 1024, 1024, 32
D = 1024
WIN, HALF = 9, 4
P = 128                       # partitions
NBLK = NB // P                # 8 row blocks
CCH = C // P                  # 8 c-chunks
DCH = D // P                  # 8 d-chunks
WWIN = 192                    # window columns per block (32-aligned segs)
SCALE = 1.0 / np.sqrt(D)      # 1/32
TC = 4                        # t-rows per stream chunk
NCH = T // TC                 # chunks per block (8)
NSTREAM = 19                  # stream pool buffers (2 blocks + 3 spares)
NPE = 3                       # residual chunks offloaded to PE+ACT

F32 = mybir.dt.float32
BF16 = mybir.dt.bfloat16
F16 = mybir.dt.float16
FP8 = mybir.dt.float8e4
NPF8 = ml_dtypes.float8_e4m3
DR = mybir.MatmulPerfMode.DoubleRow

# fp8 scale plumbing (see module docstring)
S_TOK = 1.0 / 16.0            # tokens (T-sums, std ~5.7) -> fp8
S_W = 128.0                   # projection weights W/T -> fp8
S_QK = S_TOK * S_W            # = 8: Q/K/V psum pre-scale
S_WVO = 128.0                 # fused (Wv/T)@Wo -> fp8

N_CORES = 8


def _w0(i):
    """Window start for block i; all V-block segments 32-aligned."""
    return min(max(i * P - 32, 0), NB - WWIN)


def _build_masks():
    """Per-block additive masks [NBLK, P, WWIN], pre-scaled for the
    exp (which uses scale=SCALE/S_QK^2): log(multiplicity) *
    S_QK^2/SCALE on in-band columns, -1e30 elsewhere."""
    m = np.full((NBLK, P, WWIN), -1e30, np.float32)
    for i in range(NBLK):
        w0 = _w0(i)
        for r in range(P):
            n = i * P + r
            idx = np.clip(n - HALF + np.arange(WIN), 0, NB - 1)
            u, cnt = np.unique(idx, return_counts=True)
            m[i, r, u - w0] = np.log(cnt.astype(np.float64)) * (
                S_QK * S_QK / SCALE)
    return m


_MASKS64 = _build_masks()


def _segments(i):
    """V-block segments covering window [w0, w0+WWIN) for block i as
    (blk, p0, ln, cofs): rows [p0, p0+ln) of V block `blk` correspond
    to window columns [cofs, cofs+ln).  All splits 32-aligned."""
    w0 = _w0(i)
    segs = []
    lo, hi = w0, w0 + WWIN
    for blk in range(NBLK):
        b0, b1 = blk * P, (blk + 1) * P
        s0, s1 = max(lo, b0), min(hi, b1)
        if s0 < s1:
            segs.append((blk, s0 - b0, s1 - s0, s0 - w0))
    return segs


def build_nc():
    nc = bacc.Bacc("TRN2", target_bir_lowering=False, debug=False,
                   num_devices=N_CORES)

    xb_h = nc.declare_dram_parameter("xbh", [NB, T, C], F16, isOutput=False)
    wq_d = nc.declare_dram_parameter("wq", [C, D], FP8, isOutput=False)
    wk_d = nc.declare_dram_parameter("wk", [C, D], FP8, isOutput=False)
    wvo_d = nc.declare_dram_parameter("wvo", [C, C], FP8, isOutput=False)
    posu_d = nc.declare_dram_parameter("posu", [NB, C], FP8, isOutput=False)
    mask_d = nc.declare_dram_parameter("mask", [NBLK, P, WWIN], BF16,
                                       isOutput=False)
    out_d = nc.declare_dram_parameter("out", [NB, T, C], F16, isOutput=True)

    with tile.TileContext(nc) as tc:
        _emit(nc, tc, xb_h, wq_d, wk_d, wvo_d, posu_d,
              mask_d, out_d)

    nc.compile()
    return nc


def _emit(nc, tc, xb_h, wq_d, wk_d, wvo_d, posu_d,
          mask_d, out_d):
    from contextlib import ExitStack

    with ExitStack() as ctx:
        const_pool = ctx.enter_context(tc.tile_pool(name="const", bufs=1))
        ident = const_pool.tile([P, P], BF16)
        cmasks.make_identity(nc, ident[:])
        identF = const_pool.tile([P, P], F16, tag="idF")
        cmasks.make_identity(nc, identF[:])
        ident8 = const_pool.tile([P, P], FP8, tag="id8")
        cmasks.make_identity(nc, ident8[:])

        persist = ctx.enter_context(tc.tile_pool(name="persist", bufs=1))
        kT = persist.tile([P, DCH, NB], FP8)         # K^T (d-major), *8
        # U = (V@Wo) (n-major, *8), rolling window of 4 blocks: lets
        # attention finish with a single attn^T @ U matmul instead of
        # the two-stage (attn@V)@Wo, shortening the per-slot critical
        # chain by ~6us
        uA = persist.tile([P, 4, C], FP8)

        wpool = ctx.enter_context(tc.tile_pool(name="weights", bufs=1))
        wq = wpool.tile([P, CCH, D], FP8)
        wk = wpool.tile([P, CCH, D], FP8)
        wvo = wpool.tile([P, CCH, C], FP8)

        stream = ctx.enter_context(tc.tile_pool(name="stream", bufs=NSTREAM))
        scr_pool = ctx.enter_context(tc.tile_pool(name="scratch", bufs=1))
        tok_pool = ctx.enter_context(tc.tile_pool(name="tokens", bufs=1))
        tokT_pool = ctx.enter_context(tc.tile_pool(name="tokT", bufs=2))
        mask_pool = ctx.enter_context(tc.tile_pool(name="mask", bufs=2))
        qT_pool = ctx.enter_context(tc.tile_pool(name="qT", bufs=2))
        pos_pool = ctx.enter_context(tc.tile_pool(name="pos", bufs=1))
        otok_pool = ctx.enter_context(tc.tile_pool(name="otok", bufs=2))
        tmp_pool = ctx.enter_context(tc.tile_pool(name="otokn", bufs=1))
        att_pool = ctx.enter_context(tc.tile_pool(name="attn", bufs=1))

        # PSUM: 8 banks -> psTR 1, ps512 2, ps128 2, psS 1, psR 2
        psTR = ctx.enter_context(
            tc.tile_pool(name="psTR", bufs=1, space="PSUM"))  # transposes
        ps512 = ctx.enter_context(
            tc.tile_pool(name="ps512", bufs=2, space="PSUM"))  # V/out_tok
        ps128 = ctx.enter_context(
            tc.tile_pool(name="ps128", bufs=2, space="PSUM"))  # QK/ctx
        psS = ctx.enter_context(
            tc.tile_pool(name="psS", bufs=1, space="PSUM"))   # scores
        psR = ctx.enter_context(
            tc.tile_pool(name="psR", bufs=2, space="PSUM"))   # PE residual

        def issue_loads(i):
            """Queue block i's 8 chunk loads on the SP HWDGE ring."""
            tiles = []
            for j in range(NCH):
                t = stream.tile([P, TC, C], F16, tag="xb")
                nc.sync.dma_start(
                    t[:], xb_h[i * P:(i + 1) * P, j * TC:(j + 1) * TC, :])
                tiles.append(t)
            return tiles

        def reduce_project(i, tiles):
            """T-sum block i (kept resident), transpose, project K
            (which gates attention(i-1))."""
            tok = tok_pool.tile([P, C], BF16)
            for j, t in enumerate(tiles):
                # all-2x reduction tree: out-of-place first level keeps
                # the raw xb intact for the phase-B residual
                s = scr_pool.tile([P, 2, C], F16, tag="s")
                nc.vector.tensor_tensor(
                    s[:], t[:, 0:2, :], t[:, 2:4, :],
                    op=mybir.AluOpType.add)
                with nc.allow_low_precision("tokens feed fp8 matmuls"):
                    if j == 0:
                        nc.vector.tensor_tensor(
                            tok[:], s[:, 0, :], s[:, 1, :],
                            op=mybir.AluOpType.add)
                    else:
                        nc.vector.tensor_tensor(
                            s[:, 0, :], s[:, 0, :], s[:, 1, :],
                            op=mybir.AluOpType.add)
                        nc.vector.tensor_tensor(
                            tok[:], tok[:], s[:, 0, :],
                            op=mybir.AluOpType.add)
            tokT = tokT_pool.tile([P, CCH, P], FP8)
            for cc in range(CCH):
                pt = psTR.tile([P, P], BF16, tag="tr")
                nc.tensor.transpose(pt[:], tok[:, cc * P:(cc + 1) * P],
                                    ident[:])
                nc.scalar.activation(tokT[:, cc, :], pt[:],
                                     mybir.ActivationFunctionType.Copy,
                                     scale=float(S_TOK))

            ic = slice(i * P, (i + 1) * P)
            posu8 = pos_pool.tile([P, C], FP8, tag="posu")
            nc.sync.dma_start(
                posu8[:], posu_d.rearrange("(a p) c -> p a c", p=P)[:, i, :])
            mrow = mask_pool.tile([P, WWIN], BF16, tag="m")
            nc.sync.dma_start(
                mrow[:], mask_d.rearrange("a p w -> p a w")[:, i, :])

            # K first: attention(i-1) waits on it, so this sits on the
            # per-slot critical cascade.  Plain fp8 (no DoubleRow):
            # FWL loads the 128-col stationary in ~40ns vs DR's ~218ns
            # reload per 256-row pass — at FD=128 (DR's break-even)
            # the LDW saving wins and the cascade's K segment shrinks.
            # pos is dropped from Q/K: its score contribution (~0.0004
            # in exp units) is far below the fp8 noise floor.
            for dd in range(DCH):
                ps = ps128.tile([P, P], F32, tag="qk")
                for cc in range(CCH):
                    nc.tensor.matmul(
                        ps[:], wk[:, cc, dd * P:(dd + 1) * P],
                        tokT[:, cc, :],
                        start=(cc == 0), stop=(cc == CCH - 1))
                nc.scalar.copy(kT[:, dd, ic], ps[:])
            return tokT, posu8, mrow

        def proj_u(i, tokT, posu8):
            """U(i) = tokens@((Wv/T)@Wo) + pos@Wo, fused on the host
            into one [C, C] weight — psum arrives at *8 like Q/K; the
            pos@Wo term is accumulated into the psum by an fp8
            identity matmul streaming posu, and ACT evicts to fp8.
            Emitted between attention's softmax and its out_tok matmul
            so the PE fills the softmax latency."""
            for ch in range(C // 512):
                cs = slice(ch * 512, (ch + 1) * 512)
                ups = ps512.tile([P, 512], F32, tag="v")
                for cc in range(0, CCH, 2):
                    nc.tensor.matmul(ups[:], tokT[:, cc:cc + 2, :],
                                     wvo[:, cc:cc + 2, cs],
                                     start=(cc == 0), stop=False,
                                     perf_mode=DR)
                nc.tensor.matmul(ups[:], ident8[:], posu8[:, cs],
                                 start=False, stop=True)
                with nc.allow_low_precision("uA feeds fp8 matmuls"):
                    nc.scalar.copy(uA[:, i % 4, cs], ups[:])

        def proj_q(i, tokT):
            """Q projection for block i (needed only by attention(i),
            so emitted after attention(i-1) to keep PE unblocked)."""
            qTb = qT_pool.tile([P, DCH, P], FP8)
            # plain fp8 like K: FWL's ~40ns stationary load beats DR's
            # ~218ns reload at FD=128; Q ends each slot's PE queue and
            # the ACT queue's qT casts (which head-of-line-block the
            # next slot's store dispatches) wait on it
            for dd in range(DCH):
                ps = ps128.tile([P, P], F32, tag="qk")
                for cc in range(CCH):
                    nc.tensor.matmul(
                        ps[:], wq[:, cc, dd * P:(dd + 1) * P],
                        tokT[:, cc, :],
                        start=(cc == 0), stop=(cc == CCH - 1))
                nc.scalar.copy(qTb[:, dd, :], ps[:])
            return qTb

        def attn_scores(x, qTb, mrow):
            """Banded scores + softmax + transposed attn for block x.
            The additive mask rides the PE: an identity-stationary
            matmul streams the mask row into the scores psum, so exp
            reads psum directly and the DVE is never involved."""
            w0 = _w0(x)
            segs = _segments(x)

            sc = psS.tile([P, WWIN], F32)
            for dd in range(0, DCH, 2):
                nc.tensor.matmul(sc[:], qTb[:, dd:dd + 2, :],
                                 kT[:, dd:dd + 2, w0:w0 + WWIN],
                                 start=(dd == 0), stop=False,
                                 perf_mode=DR)
            nc.tensor.matmul(sc[:], ident[:], mrow[:],
                             start=False, stop=True)
            # unnormalized exp straight to bf16; the softmax
            # denominator is applied NEXT slot in phase_b (rden recip
            # + ACT scale-copy), so this chain never queues on DVE
            attb = att_pool.tile([P, WWIN], BF16, tag="attb")
            den = att_pool.tile([P, 1], F32, tag="den", bufs=3)
            with nc.allow_low_precision("unnormalized attn weights"):
                nc.scalar.activation(attb[:], sc[:],
                                     mybir.ActivationFunctionType.Exp,
                                     scale=float(SCALE / (S_QK * S_QK)),
                                     accum_out=den[:])

            # transpose attn segments (32-aligned partition placement)
            attT = []
            for (blk, p0, ln, cofs) in segs:
                pt = psTR.tile([P, P], BF16, tag="tr")
                nc.tensor.transpose(pt[p0:p0 + ln, :],
                                    attb[:, cofs:cofs + ln], ident[:],
                                    tile_position=(0, p0))
                st = att_pool.tile([P, P], FP8, tag="attT_sb", bufs=3)
                nc.scalar.copy(st[p0:p0 + ln, :], pt[p0:p0 + ln, :])
                attT.append(st)
            return attT, den

        def attn_out(x, attT):
            """otok_un = den*out_tok: attn^T-weighted sum of U rows;
            the psum is 8*den*out_tok, evicted at 1/8 — normalization
            by 1/den happens in phase_b."""
            segs = _segments(x)
            otok = otok_pool.tile([P, C], F16, tag="otok")
            for ch in range(C // 512):
                cs = slice(ch * 512, (ch + 1) * 512)
                ops = ps512.tile([P, 512], F32, tag="v")
                for k, (blk, p0, ln, cofs) in enumerate(segs):
                    nc.tensor.matmul(
                        ops[:], attT[k][p0:p0 + ln, :],
                        uA[p0:p0 + ln, blk % 4, cs],
                        start=(k == 0), stop=(k == len(segs) - 1),
                        tile_position=(p0, 0))
                with nc.allow_low_precision("den*out_tok fits fp16"):
                    nc.scalar.activation(otok[:, cs], ops[:],
                                         mybir.ActivationFunctionType.Copy,
                                         scale=float(1.0 / S_QK))
            return otok

        def phase_b(x, tiles, otok, den, final=False):
            """Residual broadcast-add on the resident xb chunks of
            block x, then fp16 store (ACT HWDGE ring so stores never
            head-of-line-block the SP-ring loads).  First normalize:
            rden=1/den on DVE (front of the slot's DVE queue), one ACT
            per-partition-scale copy -> tmp.  Chunks 0..NPE-1 run on
            PE+ACT (identity matmuls into psum, ACT evicts in place);
            the rest are DVE broadcast-adds on the 2x path."""
            rden = att_pool.tile([P, 1], F32, tag="rden", bufs=2)
            nc.vector.reciprocal(rden[:], den[:])
            # the single tmp buffer's next grab waits on the PREVIOUS
            # phase_b's slow readers (DVE TTs); in the epilogue, borrow
            # the otok pool's free buffer instead so phase_b(7) doesn't
            # serialize behind phase_b(6)
            if final:
                tmp = otok_pool.tile([P, C], F16, tag="otok")
            else:
                tmp = tmp_pool.tile([P, C], F16)
            with nc.allow_low_precision("normalized out_tok"):
                nc.scalar.activation(tmp[:], otok[:],
                                     mybir.ActivationFunctionType.Copy,
                                     scale=rden[:])
            for j in range(NCH):
                t = tiles[j]
                if j < NPE:
                    for s8 in range(8):
                        tt, chh = divmod(s8, 2)
                        sl = slice(chh * 512, (chh + 1) * 512)
                        ps = psR.tile([P, 512], F32)
                        nc.tensor.matmul(ps[:], identF[:], t[:, tt, sl],
                                         start=True, stop=False)
                        nc.tensor.matmul(ps[:], identF[:], tmp[:, sl],
                                         start=False, stop=True)
                        nc.scalar.copy(t[:, tt, sl], ps[:])
                else:
                    nc.vector.tensor_tensor(
                        t[:], t[:],
                        tmp.unsqueeze(1).broadcast_to((P, TC, C)),
                        op=mybir.AluOpType.add)
                nc.scalar.dma_start(
                    out_d[x * P:(x + 1) * P, j * TC:(j + 1) * TC, :], t[:])

        # Software pipeline, phase B lagged TWO slots behind the loads:
        # at slot i the stores of block i-2 (whose out_tok was computed
        # during slot i-1) begin immediately, so the DMA engines never
        # wait on the current slot's attention chain.
        tiles0 = issue_loads(0)
        # weights ride the same SP ring AFTER block 0's data: xb bytes
        # start flowing at t=0; the weights land ~30us in, well before
        # their first use
        nc.sync.dma_start(wq[:], wq_d.rearrange("(a p) d -> p a d", p=P))
        nc.sync.dma_start(wk[:], wk_d.rearrange("(a p) d -> p a d", p=P))
        nc.sync.dma_start(wvo[:], wvo_d.rearrange("(a p) c -> p a c", p=P))

        prev = None        # (x, tiles, qTb, mrow): awaiting attention
        pend = None        # (x, tiles, otok, den): awaiting phase B
        for i in range(NBLK):
            if pend is not None:
                phase_b(*pend)
                pend = None
            tiles_i = tiles0 if i == 0 else issue_loads(i)
            tokT_i, posu8_i, mrow_i = reduce_project(i, tiles_i)
            if prev is not None:
                x, tiles_x, qTb_x, mrow_x = prev
                attT_x, den_x = attn_scores(x, qTb_x, mrow_x)
                proj_u(i, tokT_i, posu8_i)   # PE fills softmax latency
                otok_x = attn_out(x, attT_x)
                pend = (x, tiles_x, otok_x, den_x)
            else:
                proj_u(i, tokT_i, posu8_i)
            qTb_i = proj_q(i, tokT_i)
            prev = (i, tiles_i, qTb_i, mrow_i)
        # epilogue: kick off attention(7) BEFORE draining the pending
        # stores — block 7's window is clamped inside blocks 6..7, so
        # nothing new is needed and the tail overlaps
        x, tiles_x, qTb_x, mrow_x = prev
        attT_x, den_x = attn_scores(x, qTb_x, mrow_x)
        otok_x = attn_out(x, attT_x)
        if pend is not None:
            phase_b(*pend)
        phase_b(x, tiles_x, otok_x, den_x, final=True)


_NC = None


def _get_nc():
    global _NC
    if _NC is None:
        _NC = build_nc()
    return _NC


def _prep_in_maps(xb, Wq, Wk, Wv, Wo, pos):
    # device works t-major: [NB, T, C]
    xb_h = np.asarray(xb, np.float32).transpose(0, 1, 3, 2).astype(
        np.float16)
    wq8 = (np.asarray(Wq, np.float32) * (S_W / T)).astype(NPF8)
    wk8 = (np.asarray(Wk, np.float32) * (S_W / T)).astype(NPF8)
    wvo = (np.asarray(Wv, np.float32) / T) @ np.asarray(Wo, np.float32)
    wvo8 = (wvo * S_WVO).astype(NPF8)
    posu8 = ((np.asarray(pos, np.float32) @ np.asarray(Wo, np.float32))
             * S_QK).astype(NPF8)
    mask_h = _MASKS64.astype(ml_dtypes.bfloat16)
    in_maps = []
    for b in range(B):
        in_maps.append({
            "xbh": np.ascontiguousarray(xb_h[b]),
            "wq": wq8, "wk": wk8, "wvo": wvo8,
            "posu": posu8, "mask": mask_h,
        })
    return in_maps


def _post(res):
    out = np.stack([res.results[b]["out"] for b in range(B)], axis=0)
    # [B, NB, T, C] fp16 -> [B, NB, C, T] fp32
    return np.ascontiguousarray(out.transpose(0, 1, 3, 2)).astype(
        np.float32)


def kernel(xb, Wq, Wk, Wv, Wo, pos):
    nc = _get_nc()
    in_maps = _prep_in_maps(xb, Wq, Wk, Wv, Wo, pos)
    res = run_bass_kernel_spmd(nc, in_maps, core_ids=list(range(N_CORES)))
    return _post(res)


def run_profiled(xb, Wq, Wk, Wv, Wo, pos, **kw):
    """Like kernel(), but NTFF-profiled; returns (out, BassKernelResults)."""
    import sys, types
    if "antenv.axon_hooks" not in sys.modules:
        try:
            from trn_agent_boot.trn_boot import _ntff_profile_via_ctypes
            hook = _ntff_profile_via_ctypes('/opt/axon/libaxon_pjrt.so')
            mod = types.ModuleType("antenv.axon_hooks")
            mod.get_axon_ntff_profile_hook = lambda: hook
            mod.set_axon_ntff_profile_hook = lambda h: None
            sys.modules["antenv.axon_hooks"] = mod
            import concourse.bass_utils as bu
            bu.upload_artifacts = lambda tmpdir: f"local:{tmpdir}"
        except Exception as e:
            print(f"profiling shim unavailable: {e}")
    nc = _get_nc()
    in_maps = _prep_in_maps(xb, Wq, Wk, Wv, Wo, pos)
    res = run_bass_kernel_spmd(nc, in_maps, core_ids=list(range(N_CORES)),
                               trace=True, **kw)
    return _post(res), res
